# revision 5
# baseline (speedup 1.0000x reference)
"""CausalBiTrilinearBCNAttention Trainium2 kernel (layout-B rewrite).

Math: the network collapses to xp = x @ P (448 rank columns), causal
cumsums over 4 of the 7 rank groups, elementwise rank products, and a
final [T,128]@[128,D] projection (see P/A folding below).

This version keeps FEATURES on partitions and TOKENS on the free dim:

  xpT = P.T @ x.T        4 stationary groups of <=128 P-columns,
                         tokens stream as the moving operand
  cums = tensor_tensor_scan (DVE prefix-add along free dim, fp32 state)
  ew   = lane-aligned DVE products (P column order is chosen so every
         product pairs values living on the same partitions):
           P cols = [b3|b1 | b7|b2 | a3|a1 | a2s]
           C0 = scan(G3) = [c3@lo | c1@hi]   C0p = C0 * invc
           C1 = scan(G4) = [c7@lo | c2@hi]   C1p = C1 * invc
           g2 = a3 * (c3'*c7')  @lo          -> GT[0:64]
           g1 = a1*c1' + a2s*c2'  @hi        -> GT[64:128]
  out  = A'.T @ GT       A' = [alpha_tri*WO@U_t | WO@U_b]

so there are no PE transposes, no PE cumsum/carry chain, and the PE
stream is 64 xpT MMs + 16 final MMs, all N=512, back to back (HAM
stays warm; a dummy-MM burst warms it during the DMA lead-in).

Sharding: 8 cores = 4 batches x 2 T-halves. The T/2 carry for the
second half is folded by the host into the scan initial values
(carry = sum_t x[b,:T/2] @ P, fp32) and chained across the two
512-token device halves via the scan output's last column.
"""

import numpy as np

import concourse.bass as bass
import concourse.tile as tile
from concourse import bacc, mybir
from concourse.bass_utils import run_bass_kernel_spmd

B, T, D, R = 4, 2048, 1024, 64
TH = T // 2          # tokens per core
ND = D // 128        # 8 contraction chunks
PCOLS = 448
HW = 512             # tokens per device half
F32 = mybir.dt.float32
F16 = mybir.dt.float16

# P' column groups (each <=128 wide -> one stationary tile):
#   G3  = cols   0:128 = [b3|b1]   (cumsummed)
#   G4  = cols 128:256 = [b7|b2]   (cumsummed)
#   A1  = cols 256:384 = [a3|a1]
#   A2s = cols 384:448 = [a2s]     (M=64, written to partitions 64:128)
G3_, G4_, A1_, A2s_ = (0, 128), (128, 256), (256, 384), (384, 448)

ADD = mybir.AluOpType.add
BYP = mybir.AluOpType.bypass


def build_nc():
    nc = bacc.Bacc(None, target_bir_lowering=False)

    xT = nc.dram_tensor("xT", [D, TH], F16, kind="ExternalInput")
    P = nc.dram_tensor("P", [D, PCOLS], F16, kind="ExternalInput")
    AT = nc.dram_tensor("AT", [128, D], F16, kind="ExternalInput")
    invcT = nc.dram_tensor("invcT", [128, TH], F16, kind="ExternalInput")
    carry = nc.dram_tensor("carry", [128, 2], F32, kind="ExternalInput")
    outT = nc.dram_tensor("outT", [D, TH], F16, kind="ExternalOutput")

    xv = xT.rearrange("(k p) t -> p k t", p=128)
    Pv = P.rearrange("(k p) c -> p k c", p=128)
    ov = outT.rearrange("(k p) t -> p k t", p=128)

    with tile.TileContext(nc) as tc:
        with tc.tile_pool(name="consts", bufs=1) as consts, \
             tc.tile_pool(name="big", bufs=1) as big, \
             tc.tile_pool(name="outp", bufs=4) as outp, \
             tc.tile_pool(name="ps", bufs=1, space="PSUM") as ps:

            # ---- HAM warmup: dummy MMs bridge the DMA lead-in so the PE
            # activity window is warm when the real stream starts ----
            warm_sb = consts.tile([128, 128], F16)
            nc.gpsimd.memset(warm_sb, 0.0)
            warm_ps = ps.tile([128, 512], F32, tag="out", bufs=3)
            for _ in range(10):
                nc.tensor.matmul(warm_ps[:, 0:128], warm_sb, warm_sb,
                                 start=True, stop=True)

            # ---- SBUF tiles ----
            xT_sb = big.tile([128, ND, TH], F16)
            P_sb = consts.tile([128, ND, PCOLS], F16)
            AT_sb = consts.tile([128, D], F16)
            invcT_sb = consts.tile([128, TH], F16)
            carry_sb = consts.tile([128, 2], F32)

            C0raw = big.tile([128, 2, HW], F16)
            C1raw = big.tile([128, 2, HW], F16)
            C0p = big.tile([128, 2, HW], F16)
            C1p = big.tile([128, 2, HW], F16)
            A1sb = big.tile([128, 2, HW], F16)
            A2sb = big.tile([128, 2, HW], F16)
            t2sb = big.tile([64, 2, HW], F16)
            m1sb = big.tile([128, 2, HW], F16)
            m2sb = big.tile([128, 2, HW], F16)
            GT = big.tile([128, 2, HW], F16)

            # ---- input DMAs: just-in-time for the dk-pair-blocked h0
            # sweep, then h1 chunks; invcT/AT slotted where slack exists ----
            nc.sync.dma_start(out=P_sb[:, 0:2, :], in_=Pv[:, 0:2, :])
            nc.sync.dma_start(out=xT_sb[:, 1, 0:HW], in_=xv[:, 1, 0:HW])
            nc.sync.dma_start(out=xT_sb[:, 3, 0:HW], in_=xv[:, 3, 0:HW])
            nc.sync.dma_start(out=P_sb[:, 4:6, :], in_=Pv[:, 4:6, :])
            nc.sync.dma_start(out=xT_sb[:, 5, 0:HW], in_=xv[:, 5, 0:HW])
            nc.sync.dma_start(out=xT_sb[:, 7, 0:HW], in_=xv[:, 7, 0:HW])
            nc.sync.dma_start(out=xT_sb[:, 0:2, HW:TH], in_=xv[:, 0:2, HW:TH])
            nc.sync.dma_start(out=xT_sb[:, 2:4, HW:TH], in_=xv[:, 2:4, HW:TH])

            nc.scalar.dma_start(out=xT_sb[:, 0, 0:HW], in_=xv[:, 0, 0:HW])
            nc.scalar.dma_start(out=carry_sb, in_=carry[:, :])
            nc.scalar.dma_start(out=xT_sb[:, 2, 0:HW], in_=xv[:, 2, 0:HW])
            nc.scalar.dma_start(out=P_sb[:, 2:4, :], in_=Pv[:, 2:4, :])
            nc.scalar.dma_start(out=xT_sb[:, 4, 0:HW], in_=xv[:, 4, 0:HW])
            nc.scalar.dma_start(out=P_sb[:, 6:8, :], in_=Pv[:, 6:8, :])
            nc.scalar.dma_start(out=xT_sb[:, 6, 0:HW], in_=xv[:, 6, 0:HW])
            nc.scalar.dma_start(out=xT_sb[:, 4:6, HW:TH], in_=xv[:, 4:6, HW:TH])
            nc.scalar.dma_start(out=invcT_sb, in_=invcT[:, :])
            nc.scalar.dma_start(out=xT_sb[:, 6:8, HW:TH], in_=xv[:, 6:8, HW:TH])
            nc.scalar.dma_start(out=AT_sb, in_=AT[:, :])

            def xh(h, dk):
                return xT_sb[:, dk, h * HW:(h + 1) * HW]

            def mm(gp, cols, h, dk):
                lo, hi = cols
                out_ap = gp[64:128, :] if cols is A2s_ else gp
                nc.tensor.matmul(out_ap, P_sb[:, dk, lo:hi], xh(h, dk),
                                 start=(dk == 0), stop=(dk == ND - 1))

            # ============ half 0: dk-pair-blocked sweep (DMA-paced) ========
            g3p = ps.tile([128, HW], F32, tag="g3", bufs=2)
            g4p = ps.tile([128, HW], F32, tag="g4", bufs=1)
            a1p = ps.tile([128, HW], F32, tag="a1", bufs=1)
            a2p = ps.tile([128, HW], F32, tag="a2s", bufs=1)
            for j in range(ND // 2):
                for gp, cols in ((g3p, G3_), (g4p, G4_), (a1p, A1_),
                                 (a2p, A2s_)):
                    mm(gp, cols, 0, 2 * j)
                    mm(gp, cols, 0, 2 * j + 1)

            # h0 scans + drains + ew (V scans/products, S drains, G products)
            nc.vector.tensor_tensor_scan(C0raw[:, 0, :], g3p, invcT_sb[:, 0:HW],
                                         carry_sb[:, 0:1], ADD, BYP)
            nc.vector.tensor_tensor_scan(C1raw[:, 0, :], g4p, invcT_sb[:, 0:HW],
                                         carry_sb[:, 1:2], ADD, BYP)
            nc.scalar.copy(A1sb[:, 0, :], a1p)
            nc.scalar.copy(A2sb[64:128, 0, :], a2p[64:128, :])
            iv0 = invcT_sb[:, 0:HW]
            nc.vector.tensor_mul(C0p[:, 0, :], C0raw[:, 0, :], iv0)
            nc.vector.tensor_mul(C1p[:, 0, :], C1raw[:, 0, :], iv0)
            nc.vector.tensor_mul(m1sb[64:128, 0, :], A1sb[64:128, 0, :],
                                 C0p[64:128, 0, :])
            nc.vector.tensor_mul(m2sb[64:128, 0, :], A2sb[64:128, 0, :],
                                 C1p[64:128, 0, :])
            nc.vector.tensor_add(GT[64:128, 0, :], m1sb[64:128, 0, :],
                                 m2sb[64:128, 0, :])
            nc.gpsimd.tensor_mul(t2sb[:, 0, :], C0p[0:64, 0, :],
                                 C1p[0:64, 0, :])
            nc.gpsimd.tensor_mul(GT[0:64, 0, :], A1sb[0:64, 0, :],
                                 t2sb[:, 0, :])

            # ============ half 1: G3 then G4 (scan feeds), then h0 final ===
            g3p1 = ps.tile([128, HW], F32, tag="g3", bufs=2)
            for dk in range(ND):
                mm(g3p1, G3_, 1, dk)
            g4p1 = ps.tile([128, HW], F32, tag="g4", bufs=1)
            for dk in range(ND):
                mm(g4p1, G4_, 1, dk)

            # h1 scans emitted ahead of the h0 output drains on V
            nc.vector.tensor_tensor_scan(C0raw[:, 1, :], g3p1,
                                         invcT_sb[:, HW:TH],
                                         C0raw[:, 0, HW - 1:HW], ADD, BYP)
            nc.vector.tensor_tensor_scan(C1raw[:, 1, :], g4p1,
                                         invcT_sb[:, HW:TH],
                                         C1raw[:, 0, HW - 1:HW], ADD, BYP)

            def emit_final(h, vdks, late_vdks):
                osbs = {}
                for dk in range(ND):
                    o_ps = ps.tile([128, 512], F32, tag="out", bufs=3, name=f"ops{h}_{dk}")
                    nc.tensor.matmul(o_ps, AT_sb[:, dk * 128:(dk + 1) * 128],
                                     GT[:, h, :], start=True, stop=True)
                    if dk % 2 == 0:
                        osbs[dk] = outp.tile([128, 2, HW], F16, name=f"osb{h}_{dk}")
                    pair = osbs[dk - dk % 2]
                    if dk in vdks:
                        nc.vector.tensor_copy(pair[:, dk % 2, :], o_ps)
                    elif dk in late_vdks:
                        late_vdks[dk] = (pair, o_ps)
                    else:
                        nc.scalar.copy(pair[:, dk % 2, :], o_ps)
                return osbs

            def emit_stores(h, osbs, dks):
                for i, dk in enumerate(dks):
                    q = nc.sync if i % 2 == 0 else nc.scalar
                    q.dma_start(out=ov[:, dk:dk + 2, h * HW:(h + 1) * HW],
                                in_=osbs[dk])

            late0 = {4: None, 6: None}
            osbs0 = emit_final(0, vdks=(0, 2), late_vdks=late0)
            emit_stores(0, osbs0, (0, 2))

            a1p1 = ps.tile([128, HW], F32, tag="a1", bufs=1)
            for dk in range(ND):
                mm(a1p1, A1_, 1, dk)
            nc.scalar.copy(A1sb[:, 1, :], a1p1)

            iv1 = invcT_sb[:, HW:TH]
            nc.vector.tensor_mul(C0p[:, 1, :], C0raw[:, 1, :], iv1)
            nc.vector.tensor_mul(C1p[:, 1, :], C1raw[:, 1, :], iv1)
            # late h0 output drains once the h1 scan/products cleared V
            for dk in (4, 6):
                pair, o_ps = late0[dk]
                nc.vector.tensor_copy(pair[:, 0, :], o_ps)
            emit_stores(0, osbs0, (4, 6))
            nc.gpsimd.tensor_mul(t2sb[:, 1, :], C0p[0:64, 1, :],
                                 C1p[0:64, 1, :])
            nc.vector.tensor_mul(m1sb[64:128, 1, :], A1sb[64:128, 1, :],
                                 C0p[64:128, 1, :])
            nc.gpsimd.tensor_mul(GT[0:64, 1, :], A1sb[0:64, 1, :],
                                 t2sb[:, 1, :])

            a2p1 = ps.tile([128, HW], F32, tag="a2s", bufs=1)
            for dk in range(ND):
                mm(a2p1, A2s_, 1, dk)
            nc.scalar.copy(A2sb[64:128, 1, :], a2p1[64:128, :])
            nc.vector.tensor_mul(m2sb[64:128, 1, :], A2sb[64:128, 1, :],
                                 C1p[64:128, 1, :])
            nc.vector.tensor_add(GT[64:128, 1, :], m1sb[64:128, 1, :],
                                 m2sb[64:128, 1, :])

            late1 = {}
            osbs1 = emit_final(1, vdks=(0, 2, 4, 6), late_vdks=late1)
            emit_stores(1, osbs1, (0, 2, 4, 6))

    nc.finalize()
    return nc


_NC = None


def _get_nc():
    global _NC
    if _NC is None:
        _NC = build_nc()
    return _NC


def _fold_weights(WQ, WK, WO, Winv, U_b, V_b, W_b, U_t, V_t, W_t, X_t,
                  alpha_bi, alpha_tri):
    f8 = np.float64
    WQ, WK, WO, Winv = (np.asarray(m) for m in (WQ, WK, WO, Winv))
    U_b, V_b, W_b = (np.asarray(m) for m in (U_b, V_b, W_b))
    U_t, V_t, W_t, X_t = (np.asarray(m) for m in (U_t, V_t, W_t, X_t))
    WQt = WQ.astype(f8).T
    WKt = WK.astype(f8).T
    Winvt = Winv.astype(f8).T
    # P' columns: [b3 | b1 | b7 | b2 | a3 | a1 | a2s]
    P = np.concatenate([
        WKt @ W_t.astype(f8),                              # b3
        WKt @ W_b.astype(f8),                              # b1
        X_t.astype(f8),                                    # b7
        WKt @ (Winvt @ V_b.astype(f8)),                    # b2
        WQt @ V_t.astype(f8),                              # a3
        WQt @ V_b.astype(f8),                              # a1
        float(alpha_bi) * (WQt @ (Winvt @ W_b.astype(f8))),  # a2s
    ], axis=1)
    # A' columns: [alpha_tri*WO@U_t | WO@U_b]  (GT rows: g2 then g1)
    A = np.concatenate([
        float(alpha_tri) * (WO.astype(f8) @ U_t.astype(f8)),
        WO.astype(f8) @ U_b.astype(f8),
    ], axis=1)
    return P, A


def make_in_maps(x, P, A):
    AT = np.ascontiguousarray(A.T.astype(np.float16))
    P16 = np.ascontiguousarray(P.astype(np.float16))
    in_maps = []
    for core in range(8):
        b, h = core // 2, core % 2
        xTc = np.ascontiguousarray(x[b, h * TH:(h + 1) * TH, :].T
                                   .astype(np.float16))
        if h == 1:
            sxP = x[b, :TH, :].astype(np.float64).sum(axis=0) @ P
            carry = np.stack([sxP[0:128], sxP[128:256]], axis=1)
        else:
            carry = np.zeros((128, 2), np.float64)
        counts = np.arange(h * TH + 1, (h + 1) * TH + 1, dtype=np.float64)
        invcT = np.broadcast_to((1.0 / counts).astype(np.float16),
                                (128, TH))
        in_maps.append(dict(xT=xTc, P=P16, AT=AT,
                            invcT=np.ascontiguousarray(invcT),
                            carry=np.ascontiguousarray(
                                carry.astype(np.float32))))
    return in_maps


def kernel(x, WQ, WK, WO, Winv, U_b, V_b, W_b, bias_b,
           U_t, V_t, W_t, X_t, bias_t, alpha_bi, alpha_tri):
    x = np.asarray(x, dtype=np.float32)
    P, A = _fold_weights(WQ, WK, WO, Winv, U_b, V_b, W_b,
                         U_t, V_t, W_t, X_t, alpha_bi, alpha_tri)
    in_maps = make_in_maps(x, P, A)

    res = run_bass_kernel_spmd(_get_nc(), in_maps, core_ids=list(range(8)))

    out = np.empty((B, T, D), np.float32)
    for core in range(8):
        b, h = core // 2, core % 2
        out[b, h * TH:(h + 1) * TH, :] = \
            res.results[core]["outT"].T.astype(np.float32)

    # constant bias term (zero for the given inputs, kept for fidelity)
    bias_out = ((1.0 + float(alpha_bi)) * np.asarray(bias_b, np.float64)
                + float(alpha_tri) * np.asarray(bias_t, np.float64)) \
        @ np.asarray(WO, np.float64).T
    if np.any(bias_out):
        out += bias_out.astype(np.float32)[None, None, :]
    return out
